# revision 21
# baseline (speedup 1.0000x reference)
"""DetectionLoss Trainium2 kernel.

Contract: kernel(**inputs) takes FULL inputs (bbox_pred [16,65536,4],
conf_pred [16,65536], anchors [65536,4], target_boxes [16,32,4]) and
returns the full output (total_loss, conf_loss, bbox_loss) as f32 scalars.

Sharding: data-parallel over batch. Core k processes images 2k, 2k+1 and
emits (sum conf_l, sum bbox_l) over its two images; the host divides by 16
and sums conf+bbox for the total.

Per-image algorithm (all IoU uses are monotone compares, so we work in
log space and never divide in the hot loop):
  packed[a,t] = ln(inter+1e-38) + (31-t)*2^-20 - ln(union+EPS)
  - mp[a] = max_t packed  -> pos/neg via threshold compares (log consts)
  - per-target top-3 via the DVE top-8 instruction (exact two-level merge)
  - forced positives: packed >= thr_t accumulated with a fused is_ge/max
  - matched box: exact equality packed==mp selects argmax target (index
    perturbation reproduces jnp.argmax first-index tie-break)
  - hard-negative top-k sum via binary search on the focal threshold
"""

import os
from contextlib import ExitStack

import numpy as np

P = 128          # SBUF partitions
F = 512          # anchors per partition row
T = 32           # targets per image
BI = 2           # images per core
A = P * F        # 65536 anchors
NCORES = 8

EPS = 1e-6
TINY = 1e-38
EPSI = float(2.0 ** -20)          # index-packing epsilon (log space)
DELTA = 2e-6                      # strictness margin for iou > 0.3
LN05 = float(np.log(0.5)) + 15.0 * EPSI
LN04 = float(np.log(0.4)) + 15.0 * EPSI
LN03D = float(np.log(0.3)) + DELTA
N_SEARCH = 22                     # binary-search iterations for kth value


def _emit(nc, tc, ctx, bp_d, cp_d, an_d, tb_d, out_d, sc1_d, sc2_d, sc3_d):
    import concourse.bass as bass
    import concourse.mybir as mybir

    f32 = mybir.dt.float32
    i32 = mybir.dt.int32
    Alu = mybir.AluOpType
    Act = mybir.ActivationFunctionType
    V = nc.vector
    S = nc.scalar
    G = nc.gpsimd
    PE = nc.tensor
    ts = bass.ts

    big = ctx.enter_context(tc.tile_pool(name="big", bufs=1))
    stage = ctx.enter_context(tc.tile_pool(name="stage", bufs=1))
    per = ctx.enter_context(tc.tile_pool(name="per", bufs=1))
    tp = ctx.enter_context(tc.tile_pool(name="tp", bufs=2))
    sm = ctx.enter_context(tc.tile_pool(name="sm", bufs=2))
    ps = ctx.enter_context(tc.tile_pool(name="ps", bufs=2, space="PSUM"))

    # ---- setup: anchors ----
    AXYS = stage.tile([P, F * 4], f32, tag="AXYS")
    nc.gpsimd.dma_start(AXYS[:], an_d.rearrange("(p f) c -> p (f c)", p=P))
    AXv = AXYS[:].rearrange("p (f c) -> p c f", c=4)
    AX1 = big.tile([P, F], f32)
    AY1 = big.tile([P, F], f32)
    AX2 = big.tile([P, F], f32)
    AY2 = big.tile([P, F], f32)
    V.tensor_copy(AX1[:], AXv[:, 0])
    V.tensor_copy(AY1[:], AXv[:, 1])
    V.tensor_copy(AX2[:], AXv[:, 2])
    V.tensor_copy(AY2[:], AXv[:, 3])
    AREAA = big.tile([P, F], f32)
    aw0 = tp.tile([P, F], f32, tag="r0")
    ah0 = tp.tile([P, F], f32, tag="r1")
    V.tensor_sub(aw0[:], AX2[:], AX1[:])
    V.tensor_sub(ah0[:], AY2[:], AY1[:])
    V.tensor_mul(AREAA[:], aw0[:], ah0[:])

    # (31 - t) * EPSI row, same on all partitions
    KEPS = big.tile([P, T], f32)
    for t in range(T):
        V.memset(KEPS[:, t : t + 1], (31 - t) * EPSI)

    # per-target ln(0.3) + (31-t)*EPSI + DELTA  (on partitions 0..31)
    L03row = sm.tile([1, T], f32)
    for t in range(T):
        V.memset(L03row[:, t : t + 1], LN03D + (31 - t) * EPSI)
    nc.gpsimd.dma_start(sc3_d, L03row[:])
    L03C = big.tile([T, 1], f32)
    nc.gpsimd.dma_start(L03C[:], sc3_d.rearrange("(t one) -> t one", one=1))

    ONES = big.tile([P, 1], f32)
    V.memset(ONES[:], 1.0)
    ONESROW = big.tile([1, P], f32)
    V.memset(ONESROW[:], 1.0)
    CTINY = big.tile([P, 1], f32)
    V.memset(CTINY[:], TINY)
    OUT = big.tile([1, 2], f32)
    V.memset(OUT[:], 0.0)

    IOU = big.tile([P, T * F], f32)     # packed log-iou, t-major slices

    for b in range(BI):
        # ---- per-image loads ----
        BPS = stage.tile([P, F * 4], f32, tag="BPS", bufs=2)
        nc.gpsimd.dma_start(BPS[:], bp_d[b].rearrange("(p f) c -> p (f c)", p=P))
        BPv = BPS[:].rearrange("p (f c) -> p c f", c=4)
        BX1 = per.tile([P, F], f32, tag="BX1")
        BY1 = per.tile([P, F], f32, tag="BY1")
        BX2 = per.tile([P, F], f32, tag="BX2")
        BY2 = per.tile([P, F], f32, tag="BY2")
        V.tensor_copy(BX1[:], BPv[:, 0])
        V.tensor_copy(BY1[:], BPv[:, 1])
        V.tensor_copy(BX2[:], BPv[:, 2])
        V.tensor_copy(BY2[:], BPv[:, 3])
        CPt = per.tile([P, F], f32, tag="CPt", bufs=2)
        nc.gpsimd.dma_start(CPt[:], cp_d[b].rearrange("(p f) -> p f", p=P))

        TBrow = sm.tile([1, T * 4], f32, bufs=2)
        nc.gpsimd.dma_start(TBrow[:], tb_d[b].rearrange("t c -> (t c)")[None, :])
        TBrowV = sm.tile([1, T * 4], f32, bufs=2)
        V.tensor_copy(TBrowV[:], TBrow[:])
        TBB = per.tile([P, T * 4], f32, tag="TBB")
        tbb_ps = ps.tile([P, T * 4], f32, tag="bc_ps", name="tbb_ps")
        PE.matmul(tbb_ps[:], ONESROW[:], TBrowV[:], start=True, stop=True)
        V.tensor_copy(TBB[:], tbb_ps[:])
        TBv = TBB[:].rearrange("p (t c) -> p c t", c=4)
        TW = sm.tile([P, T], f32)
        TH = sm.tile([P, T], f32)
        ABE = per.tile([P, T], f32, tag="ABE")
        V.tensor_sub(TW[:], TBv[:, 2], TBv[:, 0])
        V.tensor_sub(TH[:], TBv[:, 3], TBv[:, 1])
        V.tensor_mul(ABE[:], TW[:], TH[:])
        V.tensor_scalar(ABE[:], ABE[:], EPS, None, op0=Alu.add)

        def tcol(tt, c):
            return TBB[:, 4 * tt + c : 4 * tt + c + 1]

        # ---- pass 1: packed log-iou ----
        for t in range(T):
            ux = tp.tile([P, F], f32, tag="ux")
            V.tensor_scalar(ux[:], AX1[:], tcol(t, 0), None, op0=Alu.max)
            dx = tp.tile([P, F], f32, tag="dx")
            V.scalar_tensor_tensor(dx[:], AX2[:], tcol(t, 2), ux[:],
                                   op0=Alu.min, op1=Alu.subtract)
            uy = tp.tile([P, F], f32, tag="uy")
            V.tensor_scalar(uy[:], AY1[:], tcol(t, 1), None, op0=Alu.max)
            dy = tp.tile([P, F], f32, tag="dy")
            V.scalar_tensor_tensor(dy[:], AY2[:], tcol(t, 3), uy[:],
                                   op0=Alu.min, op1=Alu.subtract)
            dxr = tp.tile([P, F], f32, tag="dxr")
            S.activation(dxr[:], dx[:], Act.Relu)
            dyr = tp.tile([P, F], f32, tag="dyr")
            S.activation(dyr[:], dy[:], Act.Relu)
            inter = tp.tile([P, F], f32, tag="inter")
            V.tensor_mul(inter[:], dxr[:], dyr[:])
            union = tp.tile([P, F], f32, tag="union")
            V.scalar_tensor_tensor(union[:], AREAA[:], ABE[:, t : t + 1],
                                   inter[:], op0=Alu.add, op1=Alu.subtract)
            li = tp.tile([P, F], f32, tag="li")
            S.activation(li[:], inter[:], Act.Ln, bias=CTINY[:])
            lu = tp.tile([P, F], f32, tag="lu")
            S.activation(lu[:], union[:], Act.Ln)
            V.scalar_tensor_tensor(IOU[:, ts(t, F)], li[:],
                                   KEPS[:, t : t + 1], lu[:],
                                   op0=Alu.add, op1=Alu.subtract)

        STAGE = int(os.environ.get("DETLOSS_STAGE", "9"))
        if STAGE < 2:
            V.tensor_copy(OUT[0:1, :], IOU[0:1, 0:2])
            continue
        # ---- per-anchor max over targets ----
        mp = per.tile([P, F], f32, tag="mp")
        V.tensor_reduce(mp[:], IOU[:].rearrange("p (t f) -> p f t", t=T),
                        axis=mybir.AxisListType.X, op=Alu.max)

        if STAGE < 3:
            V.tensor_copy(OUT[0:1, :], mp[0:1, 0:2])
            continue
        # ---- per-target global top-8 -> forced threshold ----
        RM8 = per.tile([P, T * 8], f32, tag="RM8")
        for t in range(T):
            V.max(RM8[:, ts(t, 8)], IOU[:, ts(t, F)])
        # bounce through DRAM to transpose [p,(t,8)] -> [t,(p,8)]
        nc.gpsimd.dma_start(
            sc1_d[b].rearrange("(t p j) -> p t j", p=P, t=T),
            RM8[:].rearrange("p (t j) -> p t j", t=T))
        T8 = per.tile([T, P * 8], f32, tag="T8", bufs=2)
        nc.gpsimd.dma_start(T8[:], sc1_d[b].rearrange("(t x) -> t x", t=T))
        G8 = sm.tile([T, 8], f32)
        V.max(G8[:], T8[:])
        mx = sm.tile([T, 1], f32)
        V.tensor_max(mx[:], G8[:, 2:3], L03C[:])
        thr = sm.tile([T, 1], f32)
        V.tensor_tensor(thr[:], mx[:], G8[:, 0:1], op=Alu.min)
        nc.gpsimd.dma_start(sc2_d[b], thr[:])
        THRrow = sm.tile([1, T], f32)
        nc.gpsimd.dma_start(THRrow[:], sc2_d[b][None, :])
        THRrowV = sm.tile([1, T], f32, bufs=2)
        V.tensor_copy(THRrowV[:], THRrow[:])
        THR = per.tile([P, T], f32, tag="THR")
        thr_ps = ps.tile([P, T], f32, tag="bc_ps", name="thr_ps")
        PE.matmul(thr_ps[:], ONESROW[:], THRrowV[:], start=True, stop=True)
        V.tensor_copy(THR[:], thr_ps[:])

        if STAGE < 4:
            V.tensor_copy(OUT[0:1, :], THR[0:1, 0:2])
            continue
        # ---- pass 2+3: forced OR + matched-box accumulation ----
        facc = per.tile([P, F], f32, tag="facc")
        V.memset(facc[:], 0.0)
        M0 = per.tile([P, F], f32, tag="M0")
        M1 = per.tile([P, F], f32, tag="M1")
        M2 = per.tile([P, F], f32, tag="M2")
        M3 = per.tile([P, F], f32, tag="M3")
        for m in (M0, M1, M2, M3):
            V.memset(m[:], 0.0)
        for t in range(T):
            V.scalar_tensor_tensor(facc[:], IOU[:, ts(t, F)],
                                   THR[:, t : t + 1], facc[:],
                                   op0=Alu.is_ge, op1=Alu.max)
            eq = tp.tile([P, F], f32, tag="eq", bufs=1)
            V.tensor_tensor(eq[:], IOU[:, ts(t, F)], mp[:], op=Alu.is_equal)
            for c, m in enumerate((M0, M1, M2, M3)):
                V.scalar_tensor_tensor(m[:], eq[:], tcol(t, c), m[:],
                                       op0=Alu.mult, op1=Alu.add)

        if STAGE < 5:
            V.tensor_copy(OUT[0:1, :], M0[0:1, 0:2])
            continue
        # ---- epilogue (register style: R0..R5 scratch [P,F] tiles) ----
        R = [per.tile([P, F], f32, tag=f"R{i}", name=f"R{i}") for i in range(5)]
        R0, R1, R2, R3, R4 = [r[:] for r in R]
        R5 = R0
        COLS = per.tile([P, 8], f32, tag="COLS")
        V.memset(COLS[:], 0.0)

        pos = per.tile([P, F], f32, tag="pos")
        V.scalar_tensor_tensor(pos[:], mp[:], LN05, facc[:],
                               op0=Alu.is_ge, op1=Alu.max,
                               accum_out=COLS[:, 0:1])
        neg = per.tile([P, F], f32, tag="neg")
        V.tensor_scalar(R0, mp[:], LN04, None, op0=Alu.is_lt)
        V.tensor_sub(R0, R0, facc[:])
        V.tensor_scalar(neg[:], R0, 0.0, None, op0=Alu.max, op1=Alu.add,
                        accum_out=COLS[:, 1:2])

        # focal loss for all anchors: fl = (0.5*pos - 0.75) * (pt-1)^2 * ln(pt)
        fl = per.tile([P, F], f32, tag="fl")
        V.tensor_mul(R0, pos[:], CPt[:])
        V.scalar_tensor_tensor(R0, R0, 2.0, CPt[:],
                               op0=Alu.mult, op1=Alu.subtract)
        V.tensor_sub(R0, R0, pos[:])                 # pt - 1
        S.activation(R1, R0, Act.Ln, bias=1.0)       # ln(pt)
        S.activation(R2, R0, Act.Square)             # (1-pt)^2
        V.tensor_mul(R1, R1, R2)
        V.tensor_scalar(R0, pos[:], 0.5, 0.75, op0=Alu.mult, op1=Alu.subtract)
        V.tensor_mul(fl[:], R0, R1)
        V.scalar_tensor_tensor(R0, fl[:], 1.0, pos[:],
                               op0=Alu.mult, op1=Alu.mult,
                               accum_out=COLS[:, 2:3])
        ns = per.tile([P, F], f32, tag="ns")
        V.tensor_mul(ns[:], fl[:], neg[:])

        # giou + l1 on (bbox_pred, matched)
        V.tensor_max(R0, BX1[:], M0[:])
        V.tensor_tensor(R1, BX2[:], M2[:], op=Alu.min)
        V.tensor_sub(R1, R1, R0)
        V.tensor_scalar(R1, R1, 0.0, None, op0=Alu.max)   # dxc
        V.tensor_max(R0, BY1[:], M1[:])
        V.tensor_tensor(R2, BY2[:], M3[:], op=Alu.min)
        V.tensor_sub(R2, R2, R0)
        V.tensor_scalar(R2, R2, 0.0, None, op0=Alu.max)   # dyc
        V.tensor_mul(R1, R1, R2)                          # bint
        V.tensor_sub(R0, BX2[:], BX1[:])
        V.tensor_sub(R2, BY2[:], BY1[:])
        V.tensor_mul(R0, R0, R2)                          # areab
        V.tensor_sub(R2, M2[:], M0[:])
        V.tensor_sub(R3, M3[:], M1[:])
        V.tensor_mul(R2, R2, R3)                          # aream
        V.tensor_add(R0, R0, R2)
        V.tensor_sub(R0, R0, R1)                          # uni
        V.tensor_max(R2, BX2[:], M2[:])
        V.tensor_tensor(R3, BX1[:], M0[:], op=Alu.min)
        V.tensor_sub(R2, R2, R3)                          # dex
        V.tensor_max(R3, BY2[:], M3[:])
        V.tensor_tensor(R4, BY1[:], M1[:], op=Alu.min)
        V.tensor_sub(R3, R3, R4)                          # dey
        V.tensor_mul(R2, R2, R3)                          # enc
        V.tensor_scalar(R3, R0, EPS, None, op0=Alu.add)
        V.reciprocal(R3, R3)                              # 1/(uni+eps)
        V.tensor_mul(R1, R1, R3)                          # ioub
        V.tensor_scalar(R3, R2, EPS, None, op0=Alu.add)
        V.reciprocal(R3, R3)                              # 1/(enc+eps)
        V.tensor_sub(R2, R2, R0)                          # enc-uni
        V.tensor_mul(R2, R2, R3)
        V.tensor_sub(R2, R2, R1)                          # pb0 = (enc-uni)/(enc+e) - iou
        # L1 sum
        V.tensor_sub(R0, BX1[:], M0[:])
        S.activation(R0, R0, Act.Abs)
        V.tensor_sub(R1, BY1[:], M1[:])
        S.activation(R1, R1, Act.Abs)
        V.tensor_add(R0, R0, R1)
        V.tensor_sub(R1, BX2[:], M2[:])
        S.activation(R1, R1, Act.Abs)
        V.tensor_sub(R3, BY2[:], M3[:])
        S.activation(R3, R3, Act.Abs)
        V.tensor_add(R1, R1, R3)
        V.tensor_add(R0, R0, R1)                          # l1 sum
        V.scalar_tensor_tensor(R2, R0, 0.125, R2, op0=Alu.mult, op1=Alu.add)
        V.tensor_scalar(R2, R2, 1.0, None, op0=Alu.add)   # per_box
        V.scalar_tensor_tensor(R0, R2, 1.0, pos[:],
                               op0=Alu.mult, op1=Alu.mult,
                               accum_out=COLS[:, 3:4])

        # ---- reduce COLS across partitions ----
        pssum = ps.tile([1, 8], f32, tag="acc_ps", name="pssum")
        PE.matmul(pssum[:], ONES[:], COLS[:], start=True, stop=True)
        SU = sm.tile([1, 8], f32)
        V.tensor_copy(SU[:], pssum[:])
        npos = SU[:, 0:1]
        nneg = SU[:, 1:2]
        pfs = SU[:, 2:3]
        pbs = SU[:, 3:4]

        # k_neg = npos>0 ? min(nneg, 3*npos) : min(nneg, 100)   (on [1,1])
        k1 = sm.tile([1, 1], f32)
        V.tensor_scalar(k1[:], npos, 3.0, None, op0=Alu.mult)
        V.tensor_tensor(k1[:], k1[:], nneg, op=Alu.min)
        k0 = sm.tile([1, 1], f32)
        V.tensor_scalar(k0[:], nneg, 100.0, None, op0=Alu.min)
        zz = sm.tile([1, 1], f32)
        V.tensor_scalar(zz[:], npos, 0.5, None, op0=Alu.is_lt)
        kd = sm.tile([1, 1], f32)
        V.tensor_sub(kd[:], k0[:], k1[:])
        V.tensor_mul(kd[:], kd[:], zz[:])
        kk = per.tile([1, 1], f32, tag="kk")
        V.tensor_add(kk[:], k1[:], kd[:])

        if STAGE < 6:
            V.tensor_copy(OUT[0:1, 0:1], kk[0:1, 0:1])
            continue
        # ---- binary search for k-th largest negative focal ----
        lo = per.tile([1, 1], f32, tag="lo")
        hi = per.tile([1, 1], f32, tag="hi")
        V.memset(lo[:], 0.0)
        V.memset(hi[:], 4.0)
        for it in range(N_SEARCH):
            tau = sm.tile([1, 1], f32)
            V.tensor_add(tau[:], lo[:], hi[:])
            V.tensor_scalar(tau[:], tau[:], 0.5, None, op0=Alu.mult)
            tau_ps = ps.tile([P, 1], f32, tag="bc_ps", name="tau_ps")
            PE.matmul(tau_ps[:], ONESROW[:], tau[:], start=True, stop=True)
            taub = sm.tile([P, 1], f32)
            V.tensor_copy(taub[:], tau_ps[:])
            cntc = sm.tile([P, 1], f32)
            V.tensor_scalar(R5, ns[:], taub[:], None, op0=Alu.is_gt,
                            op1=Alu.add, accum_out=cntc[:])
            psc = ps.tile([1, 1], f32, tag="acc_ps", name="psc")
            PE.matmul(psc[:], ONES[:], cntc[:], start=True, stop=True)
            gg = sm.tile([1, 1], f32)
            V.tensor_tensor(gg[:], psc[:], kk[:], op=Alu.is_ge)
            d1 = sm.tile([1, 1], f32)
            V.tensor_sub(d1[:], tau[:], lo[:])
            V.tensor_mul(d1[:], d1[:], gg[:])
            V.tensor_add(lo[:], lo[:], d1[:])
            d2 = sm.tile([1, 1], f32)
            V.tensor_sub(d2[:], hi[:], tau[:])
            V.tensor_mul(d2[:], d2[:], gg[:])
            V.tensor_add(hi[:], tau[:], d2[:])

        # final count + sum above lo
        lo_ps = ps.tile([P, 1], f32, tag="bc_ps", name="lo_ps")
        PE.matmul(lo_ps[:], ONESROW[:], lo[:], start=True, stop=True)
        lob = sm.tile([P, 1], f32)
        V.tensor_copy(lob[:], lo_ps[:])
        cnt2 = sm.tile([P, 2], f32)
        V.tensor_scalar(R5, ns[:], lob[:], None, op0=Alu.is_gt,
                        op1=Alu.add, accum_out=cnt2[:, 0:1])
        V.scalar_tensor_tensor(R5, ns[:], lob[:], ns[:],
                               op0=Alu.is_gt, op1=Alu.mult,
                               accum_out=cnt2[:, 1:2])
        ps2 = ps.tile([1, 2], f32, tag="acc_ps", name="ps2")
        PE.matmul(ps2[:], ONES[:], cnt2[:], start=True, stop=True)
        C2 = sm.tile([1, 2], f32)
        V.tensor_copy(C2[:], ps2[:])
        nf = C2[:, 0:1]
        sf = C2[:, 1:2]

        # conf_loss = (pfs + sf + (k - nf)*lo) / max(npos + k, 1)
        kmn = sm.tile([1, 1], f32)
        V.tensor_sub(kmn[:], kk[:], nf)
        V.tensor_mul(kmn[:], kmn[:], lo[:])
        cnum = sm.tile([1, 1], f32)
        V.tensor_add(cnum[:], pfs, sf)
        V.tensor_add(cnum[:], cnum[:], kmn[:])
        den = sm.tile([1, 1], f32)
        V.tensor_add(den[:], npos, kk[:])
        V.tensor_scalar(den[:], den[:], 1.0, None, op0=Alu.max)
        rden = sm.tile([1, 1], f32)
        V.reciprocal(rden[:], den[:])
        confl = sm.tile([1, 1], f32)
        V.tensor_mul(confl[:], cnum[:], rden[:])

        # bbox_loss = npos>0 ? pbs / max(npos,1) : 0
        np1 = sm.tile([1, 1], f32)
        V.tensor_scalar(np1[:], npos, 1.0, None, op0=Alu.max)
        rnp = sm.tile([1, 1], f32)
        V.reciprocal(rnp[:], np1[:])
        bl0 = sm.tile([1, 1], f32)
        V.tensor_mul(bl0[:], pbs, rnp[:])
        zp = sm.tile([1, 1], f32)
        V.tensor_scalar(zp[:], npos, 0.5, None, op0=Alu.is_gt)
        bbl = sm.tile([1, 1], f32)
        V.tensor_mul(bbl[:], bl0[:], zp[:])

        V.tensor_add(OUT[0:1, 0:1], OUT[0:1, 0:1], confl[:])
        V.tensor_add(OUT[0:1, 1:2], OUT[0:1, 1:2], bbl[:])

    nc.gpsimd.dma_start(out_d, OUT[0:1, :])


def build():
    import concourse.bacc as bacc
    import concourse.mybir as mybir
    import concourse.tile as tile

    f32 = mybir.dt.float32
    nc = bacc.Bacc("TRN2", target_bir_lowering=False, debug=False)
    bp_d = nc.dram_tensor("bp", [BI, A, 4], f32, kind="ExternalInput")
    cp_d = nc.dram_tensor("cp", [BI, A], f32, kind="ExternalInput")
    an_d = nc.dram_tensor("an", [A, 4], f32, kind="ExternalInput")
    tb_d = nc.dram_tensor("tb", [BI, T, 4], f32, kind="ExternalInput")
    out_d = nc.dram_tensor("out", [2], f32, kind="ExternalOutput")
    sc1_d = nc.dram_tensor("scratch1", [BI, T * P * 8], f32)
    sc2_d = nc.dram_tensor("scratch2", [BI, T], f32)
    sc3_d = nc.dram_tensor("scratch3", [T], f32)
    with tile.TileContext(nc) as tc:
        with ExitStack() as ctx:
            _emit(nc, tc, ctx, bp_d.ap(), cp_d.ap(), an_d.ap(), tb_d.ap(),
                  out_d.ap(), sc1_d.ap(), sc2_d.ap(), sc3_d.ap())
    nc.compile()
    return nc


def kernel(bbox_pred, conf_pred, anchors, target_boxes):
    from concourse.bass_utils import run_bass_kernel_spmd

    bp = np.ascontiguousarray(np.asarray(bbox_pred, dtype=np.float32))
    cp = np.ascontiguousarray(np.asarray(conf_pred, dtype=np.float32))
    an = np.ascontiguousarray(np.asarray(anchors, dtype=np.float32))
    tb = np.ascontiguousarray(np.asarray(target_boxes, dtype=np.float32))

    nc = build()
    in_maps = []
    for k in range(NCORES):
        sl = slice(BI * k, BI * (k + 1))
        in_maps.append({"bp": bp[sl], "cp": cp[sl], "an": an, "tb": tb[sl]})
    trace = bool(int(os.environ.get("DETLOSS_TRACE", "0")))
    res = run_bass_kernel_spmd(nc, in_maps, list(range(NCORES)), trace=trace)
    partials = np.stack([res.results[k]["out"] for k in range(NCORES)])  # [8,2]
    conf = np.float32(partials[:, 0].sum() / 16.0)
    bbox = np.float32(partials[:, 1].sum() / 16.0)
    total = np.float32(conf + bbox)
    if trace:
        kernel.last_exec_time_ns = res.exec_time_ns
    return (total, conf, bbox)
